# revision 1
# baseline (speedup 1.0000x reference)
"""Trainium2 Bass kernel for Exaone4-style GQA attention block (T=2048, HID=4096,
H=32 q-heads, HK=8 kv-heads, D=128, sliding window 1023, QK-RMSNorm + NeoX RoPE).

Sharding: tensor-parallel over heads across 8 NeuronCores. Core m owns q-heads
[4m, 4m+4) and kv-head m (GQA group-aligned), plus the matching o_proj column
slice; per-core partial outputs are summed on the host (the all-reduce).

Device layout notes:
 - qkv projection is computed transposed ([feature, t]) so attention works in
   the S^T = K^T.T @ Q^T layout; softmax sums over the partition axis are done
   with ones-vector matmuls on the PE, and PV consumes exp(S^T) directly.
 - RMSNorm scale and RoPE are fused via host-precomputed [128, T] cos/sin
   tables (norm weights + 1/sqrt(D) folded in); the partition-half rotation
   for RoPE uses SBUF->SBUF DMA.
 - All large matmuls use bf16 operands with fp32 PSUM accumulation.
"""

import sys

import numpy as np

if "/opt/trn_rl_repo" not in sys.path:
    sys.path.insert(0, "/opt/trn_rl_repo")

import ml_dtypes

BF16 = ml_dtypes.bfloat16

HID = 4096
H = 32
HK = 8
D = 128
WIN = 1023
THETA = 1000000.0
EPS = 1e-6
SCALE = D ** -0.5
M = 8            # cores
QH = H // M      # q heads per core (4)
NJ = QH + 2      # j-blocks in qkv^T output (4 q + 1 k + 1 v)
TB = 512         # t free-dim block
NEG = -1.0e30

_PROG_CACHE = {}


def _build_program(T):
    """Build the (single-core SPMD) Bass program for sequence length T."""
    from contextlib import ExitStack

    import concourse.bass as bass  # noqa: F401
    import concourse.tile as tile
    from concourse import bacc, mybir
    from concourse.masks import make_identity

    f32 = mybir.dt.float32
    bf = mybir.dt.bfloat16

    NT = T // TB          # number of t blocks
    NC = HID // 128       # contraction chunks
    NOB = HID // 128      # output row blocks

    nc = bacc.Bacc(
        "TRN2",
        target_bir_lowering=False,
        debug=False,
        enable_asserts=False,
        num_devices=M,
    )

    # x pre-tiled on host: block (tb, cq) = [128, 4*TB], 4 c-chunks interleaved
    # per partition row (4KB contiguous per partition per DMA)
    xT_h = nc.dram_tensor(
        "xT", [(T // TB) * (HID // 512) * 128, 4 * TB], bf, kind="ExternalInput"
    )
    wq_h = nc.dram_tensor("wqkvT", [HID, NJ * 128], bf, kind="ExternalInput")
    wo_h = nc.dram_tensor("woT", [QH * 128, HID], bf, kind="ExternalInput")
    cwq_h = nc.dram_tensor("cwq", [128, T], bf, kind="ExternalInput")
    swq_h = nc.dram_tensor("swq", [128, T], bf, kind="ExternalInput")
    cwk_h = nc.dram_tensor("cwk", [128, T], bf, kind="ExternalInput")
    swk_h = nc.dram_tensor("swk", [128, T], bf, kind="ExternalInput")
    maskd_h = nc.dram_tensor("maskd", [128, 128], f32, kind="ExternalInput")
    maskw_h = nc.dram_tensor("maskw", [128, 128], f32, kind="ExternalInput")
    # out pre-tiled: block (tb, obp) = [128, 2*TB] (ob pairs interleaved per row)
    outT_h = nc.dram_tensor(
        "outT", [(T // TB) * (HID // 256) * 128, 2 * TB], bf, kind="ExternalOutput"
    )

    xTr = xT_h.ap().rearrange("(b p) u -> b p u", p=128)
    wqr = wq_h.ap().rearrange("(c p) j -> p c j", p=128)
    wor = wo_h.ap().rearrange("(jc p) o -> p jc o", p=128)
    outr = outT_h.ap().rearrange("(b p) u -> b p u", p=128)

    mult = mybir.AluOpType.mult
    add = mybir.AluOpType.add
    Exp = mybir.ActivationFunctionType.Exp
    Sqrt = mybir.ActivationFunctionType.Sqrt

    with tile.TileContext(nc) as tc, ExitStack() as ctx:
        singles = ctx.enter_context(tc.tile_pool(name="singles", bufs=1))
        persist = ctx.enter_context(tc.tile_pool(name="persist", bufs=1))
        xpool = ctx.enter_context(tc.tile_pool(name="xpool", bufs=6))
        stpool = ctx.enter_context(tc.tile_pool(name="stpool", bufs=1))
        ropep = ctx.enter_context(tc.tile_pool(name="ropep", bufs=2))
        espool = ctx.enter_context(tc.tile_pool(name="espool", bufs=4))
        outp = ctx.enter_context(tc.tile_pool(name="outp", bufs=2))
        smallp = ctx.enter_context(tc.tile_pool(name="smallp", bufs=2))
        # PSUM: every tile is <= one bank; a single tag with 8 rotating slots
        # covers all 8 banks and lets phases overlap freely.
        psum = ctx.enter_context(tc.tile_pool(name="psum", bufs=8, space="PSUM"))
        drp = ctx.enter_context(tc.tile_pool(name="drp", bufs=4, space="DRAM"))

        def bcast_row(src_row, tag):
            """Broadcast a [1, TB] sbuf row to a [128, TB] sbuf tile.

            SBUF sources cannot have zero partition step in a DMA, so bounce
            through a DRAM scratch row and broadcast-read it back."""
            drs = drp.tile([1, TB], f32, name=f"drs_{tag}", tag=f"dr_{tag}")
            nc.gpsimd.dma_start(drs, src_row)
            dst = ropep.tile([128, TB], f32, name=f"bc_{tag}", tag=tag)
            nc.gpsimd.dma_start(dst, drs.to_broadcast([128, TB]))
            return dst

        # ---- resident constants -------------------------------------------
        w_sb = singles.tile([128, NC, NJ * 128], bf)
        nc.sync.dma_start(w_sb, wqr)
        cwq_sb = singles.tile([128, T], bf)
        nc.sync.dma_start(cwq_sb, cwq_h.ap())
        swq_sb = singles.tile([128, T], bf)
        nc.sync.dma_start(swq_sb, swq_h.ap())
        cwk_sb = singles.tile([128, T], bf)
        nc.sync.dma_start(cwk_sb, cwk_h.ap())
        swk_sb = singles.tile([128, T], bf)
        nc.sync.dma_start(swk_sb, swk_h.ap())
        maskd_sb = singles.tile([128, 128], f32)
        nc.sync.dma_start(maskd_sb, maskd_h.ap())
        maskw_sb = singles.tile([128, 128], f32)
        nc.sync.dma_start(maskw_sb, maskw_h.ap())
        wo_sb = singles.tile([128, QH, HID], bf)
        nc.sync.dma_start(wo_sb, wor)
        ident = singles.tile([128, 128], bf)
        make_identity(nc, ident)
        ones_bf = singles.tile([128, 1], bf)
        nc.vector.memset(ones_bf, 1.0)
        eps_sb = singles.tile([128, 1], f32)
        nc.vector.memset(eps_sb, EPS)

        # ---- persistent activations ---------------------------------------
        qT = persist.tile([128, QH, T], bf)     # rope'd+normed q^T
        kT = persist.tile([128, T], bf)         # rope'd+normed k^T
        Vt = persist.tile([128, T // 128, 128], bf)  # v in [s, d] layout

        def phase_a(tb):
            """qkv projection + rmsnorm + rope for t block tb."""
            t0 = tb * TB
            ts_ = slice(t0, t0 + TB)
            stage = stpool.tile(
                [128, NJ, TB], bf, tag="stage", bufs=2, name=f"stage_{tb}"
            )
            for g in range(2):
                js = range(3 * g, 3 * g + 3)
                ps_g = [
                    psum.tile([128, TB], f32, name=f"psqkv_{tb}_{j}", tag="bank")
                    for j in js
                ]
                for cq in range(NC // 4):
                    xc = xpool.tile([128, 4, TB], bf, tag="xc", name=f"xc_{tb}_{g}_{cq}")
                    nc.sync.dma_start(
                        xc,
                        xTr[tb * (NC // 4) + cq].rearrange("p (ci u) -> p ci u", u=TB),
                    )
                    for ci in range(4):
                        c = 4 * cq + ci
                        for ji, j in enumerate(js):
                            nc.tensor.matmul(
                                ps_g[ji],
                                lhsT=w_sb[:, c, j * 128 : (j + 1) * 128],
                                rhs=xc[:, ci, :],
                                start=(c == 0),
                                stop=(c == NC - 1),
                            )
                for ji, j in enumerate(js):
                    nc.vector.tensor_copy(stage[:, j], ps_g[ji])

            # v: transpose [d, t] -> [s, d] blocks via PE
            for u in range(TB // 128):
                ps_t = psum.tile([128, 128], bf, name=f"pst_{tb}_{u}", tag="bank")
                nc.tensor.transpose(ps_t, stage[:, QH + 1, u * 128 : (u + 1) * 128], ident)
                nc.any.tensor_copy(Vt[:, tb * (TB // 128) + u, :], ps_t)

            # rms scale: 1/sqrt(mean(x^2) + eps) per j-block via ones-matmul
            scls = []
            for j in range(QH + 1):
                sq = stpool.tile([128, TB], bf, tag="sq", bufs=2, name=f"sq_{tb}_{j}")
                nc.vector.tensor_tensor(sq, stage[:, j], stage[:, j], mult)
                ps_ss = psum.tile([1, TB], f32, name=f"psss_{tb}_{j}", tag="bank")
                nc.tensor.matmul(ps_ss, lhsT=ones_bf, rhs=sq, start=True, stop=True)
                rms = smallp.tile([1, TB], f32, tag="rms", name=f"rms_{tb}_{j}")
                nc.scalar.activation(rms, ps_ss, Sqrt, bias=eps_sb[0:1, :], scale=1.0 / D)
                scl = smallp.tile([1, TB], f32, tag="scl", name=f"scl_{tb}_{j}")
                nc.vector.reciprocal_approx_fast(scl, rms)
                scls.append(scl)

            for j in range(QH + 1):
                sclb = bcast_row(scls[j], "sclb")
                qn = ropep.tile([128, TB], f32, tag="qn", name=f"qn_{tb}_{j}")
                nc.vector.tensor_tensor(qn, stage[:, j], sclb, mult)
                qrot = ropep.tile([128, TB], f32, tag="qrot", name=f"qrot_{tb}_{j}")
                nc.gpsimd.dma_start(qrot[0:64, :], qn[64:128, :])
                nc.gpsimd.dma_start(qrot[64:128, :], qn[0:64, :])
                cw = cwq_sb if j < QH else cwk_sb
                sw = swq_sb if j < QH else swk_sb
                b_t = ropep.tile([128, TB], f32, tag="b_t", name=f"bt_{tb}_{j}")
                nc.vector.tensor_tensor(b_t, qrot, sw[:, ts_], mult)
                nc.vector.tensor_tensor(qn, qn, cw[:, ts_], mult)
                dest = qT[:, j, ts_] if j < QH else kT[:, ts_]
                nc.vector.tensor_tensor(dest, qn, b_t, add)

        attnTs = {}

        def phase_b(tb):
            """attention for t block tb (attnT kept for phase_c)."""
            t0 = tb * TB
            ts_ = slice(t0, t0 + TB)
            # o = sb - 4*tb; o=0 (full col range) goes FIRST so the
            # start=True PV/rowsum matmuls cover the whole bank; later
            # partial-range matmuls accumulate onto uniformly-written bytes
            # (CoreSim requires this; matches HW has_written semantics).
            obs = [0] + [o for o in range(-8, 4) if o != 0 and 4 * tb + o >= 0]
            attnT = outp.tile([128, QH, TB], bf, tag="attnT", name=f"attnT_{tb}")
            attnTs[tb] = attnT
            for hp in range(QH // 2):
                heads = (2 * hp, 2 * hp + 1)
                pvs = {
                    h: psum.tile([128, TB], f32, name=f"pspv_{tb}_{h}", tag="bank")
                    for h in heads
                }
                rs = {
                    h: psum.tile([1, TB], f32, name=f"psr_{tb}_{h}", tag="bank")
                    for h in heads
                }
                for oi, o in enumerate(obs):
                    sb = 4 * tb + o
                    if o >= 0:
                        c0, c1 = 128 * o, TB
                    elif o >= -4:
                        c0, c1 = 0, TB
                    else:
                        c0, c1 = 0, 128 * (o + 9)
                    first = oi == 0
                    last = oi == len(obs) - 1
                    for h in heads:
                        ps_s = psum.tile(
                            [128, TB], f32, name=f"pss_{tb}_{h}_{oi}", tag="bank"
                        )
                        nc.tensor.matmul(
                            ps_s[:, c0:c1],
                            lhsT=kT[:, sb * 128 : (sb + 1) * 128],
                            rhs=qT[:, h, t0 + c0 : t0 + c1],
                            start=True,
                            stop=True,
                        )
                        if o >= 0:  # causal strip at cols [128o, 128o+128)
                            u0 = 128 * o
                            nc.vector.tensor_tensor(
                                ps_s[:, u0 : u0 + 128], ps_s[:, u0 : u0 + 128],
                                maskd_sb, add,
                            )
                        elif o <= -5:  # window strip
                            u0 = 128 * (o + 8)
                            nc.vector.tensor_tensor(
                                ps_s[:, u0 : u0 + 128], ps_s[:, u0 : u0 + 128],
                                maskw_sb, add,
                            )
                        es = espool.tile(
                            [128, TB], bf, tag="es", name=f"es_{tb}_{h}_{oi}"
                        )
                        nc.scalar.activation(es[:, c0:c1], ps_s[:, c0:c1], Exp)
                        nc.tensor.matmul(
                            pvs[h][:, c0:c1],
                            lhsT=Vt[:, sb, :],
                            rhs=es[:, c0:c1],
                            start=first,
                            stop=last,
                            skip_group_check=True,
                        )
                        nc.tensor.matmul(
                            rs[h][:, c0:c1],
                            lhsT=ones_bf,
                            rhs=es[:, c0:c1],
                            start=first,
                            stop=last,
                            skip_group_check=True,
                        )
                for h in heads:
                    # normalize: attnT[:, h] = pv * (1/rowsum) broadcast
                    rsum = smallp.tile([1, TB], f32, tag="rsum", name=f"rsum_{tb}_{h}")
                    nc.vector.tensor_copy(rsum, rs[h])
                    nc.vector.reciprocal_approx_fast(rsum, rsum)
                    rb = bcast_row(rsum, "sclb")
                    nc.vector.tensor_tensor(attnT[:, h, :], pvs[h], rb, mult)

        def phase_c(tb):
            """o_proj partial for t block tb (store ob pairs as one DMA)."""
            attnT = attnTs.pop(tb)
            for obp in range(NOB // 2):
                o_st = outp.tile(
                    [128, 2, TB], bf, tag="o_st", bufs=3, name=f"ost_{tb}_{obp}"
                )
                for oi in range(2):
                    ob = 2 * obp + oi
                    ps_o = psum.tile([128, TB], f32, name=f"pso_{tb}_{ob}", tag="bank")
                    for jc in range(QH):
                        nc.tensor.matmul(
                            ps_o,
                            lhsT=wo_sb[:, jc, ob * 128 : (ob + 1) * 128],
                            rhs=attnT[:, jc, :],
                            start=(jc == 0),
                            stop=(jc == QH - 1),
                        )
                    nc.vector.tensor_copy(o_st[:, oi, :], ps_o)
                nc.gpsimd.dma_start(
                    outr[tb * (NOB // 2) + obp].rearrange("p (oi u) -> p oi u", u=TB),
                    o_st,
                )

        # Software pipeline: emit phase A one t-block ahead (next block's qkv
        # runs while this block's rope/softmax chains sit on DVE/ACT/DMA),
        # and phase C one block BEHIND (o_proj has no exp dependency, so it
        # fills the PE during the next attention phase's softmax stalls).
        phase_a(0)
        for tb in range(NT):
            if tb + 1 < NT:
                phase_a(tb + 1)
            phase_b(tb)
            if tb >= 1:
                phase_c(tb - 1)
        phase_c(NT - 1)

    nc.compile()
    return nc


def _get_program(T):
    if T not in _PROG_CACHE:
        _PROG_CACHE[T] = _build_program(T)
    return _PROG_CACHE[T]


def _host_prep(positions, hidden_states, wqkv, wo, q_norm_w, k_norm_w):
    """Build the 8 per-core input maps (host-side sharding + table prep)."""
    T = hidden_states.shape[0]
    pos = np.asarray(positions).astype(np.float64)
    hs = np.asarray(hidden_states, dtype=np.float32)
    wqkv = np.asarray(wqkv, dtype=np.float32)
    wo = np.asarray(wo, dtype=np.float32)
    qw = np.asarray(q_norm_w, dtype=np.float64)
    kw = np.asarray(k_norm_w, dtype=np.float64)

    half = D // 2
    inv_freq = 1.0 / (THETA ** (np.arange(0, D, 2, dtype=np.float64) / D))  # [64]
    th = pos[:, None] * inv_freq[None, :]          # [T, 64]
    cos = np.cos(th).T                             # [64, T] float64
    sin = np.sin(th).T

    def tables(w, scale):
        cw = np.empty((D, T), np.float64)
        sw = np.empty((D, T), np.float64)
        cw[:half] = cos * (w[:half, None] * scale)
        cw[half:] = cos * (w[half:, None] * scale)
        # out[d<64] = qn[d]*w[d]*cos - qn[d+64]*w[d+64]*sin  (rot reads qn[d+64])
        sw[:half] = -sin * (w[half:, None] * scale)
        # out[d>=64] = qn[d]*w[d]*cos + qn[d-64]*w[d-64]*sin
        sw[half:] = sin * (w[:half, None] * scale)
        return cw.astype(BF16), sw.astype(BF16)

    cwq, swq = tables(qw, SCALE)
    cwk, swk = tables(kw, 1.0)

    si = np.arange(128)[:, None]
    ui = np.arange(128)[None, :]
    maskd = np.where(ui >= si, 0.0, NEG).astype(np.float32)
    maskw = np.where(ui < si, 0.0, NEG).astype(np.float32)

    # tiled layout: block (tb, cq) = [128, 4*TB]; row p holds c-chunks
    # 4cq..4cq+3 back to back (4KB contiguous per partition)
    NTb, NCq = T // TB, HID // 512
    xT = np.ascontiguousarray(
        hs.T.reshape(NCq, 4, 128, NTb, TB)
        .transpose(3, 0, 2, 1, 4)
        .reshape(NTb * NCq * 128, 4 * TB)
    ).astype(BF16)

    in_maps = []
    for m in range(M):
        wq_m = wqkv[m * QH * D : (m + 1) * QH * D]            # [512, HID]
        wk_m = wqkv[H * D + m * D : H * D + (m + 1) * D]      # [128, HID]
        wv_m = wqkv[(H + HK) * D + m * D : (H + HK) * D + (m + 1) * D]
        wqkvT_m = np.ascontiguousarray(
            np.concatenate([wq_m, wk_m, wv_m], axis=0).T
        ).astype(BF16)                                        # [HID, 768]
        woT_m = np.ascontiguousarray(
            wo[:, m * QH * D : (m + 1) * QH * D].T
        ).astype(BF16)                                        # [512, HID]
        in_maps.append(
            {
                "xT": xT,
                "wqkvT": wqkvT_m,
                "woT": woT_m,
                "cwq": cwq,
                "swq": swq,
                "cwk": cwk,
                "swk": swk,
                "maskd": maskd,
                "maskw": maskw,
            }
        )
    return in_maps


def _run(in_maps, T, trace=False):
    from concourse import bass_utils

    nc = _get_program(T)
    res = bass_utils.run_bass_kernel_spmd(
        nc, in_maps, core_ids=list(range(M)), trace=trace
    )
    return res


def kernel(positions, hidden_states, wqkv, wo, q_norm_w, k_norm_w, _trace=False):
    T = hidden_states.shape[0]
    in_maps = _host_prep(positions, hidden_states, wqkv, wo, q_norm_w, k_norm_w)
    res = _run(in_maps, T, trace=_trace)
    NTb, NOBp = T // TB, HID // 256
    acc = np.zeros((NTb, NOBp, 128, 2, TB), np.float64)
    for r in res.results:
        acc += r["outT"].astype(np.float64).reshape(NTb, NOBp, 128, 2, TB)
    # untile: out[t, o] with o = (2*obp + oi)*128 + p, t = tb*TB + u
    out = np.ascontiguousarray(
        acc.transpose(0, 4, 1, 3, 2).reshape(T, HID)
    ).astype(np.float32)
    kernel._last_results = res
    return out



# revision 6
# speedup vs baseline: 1.0970x; 1.0970x over previous
"""Trainium2 Bass kernel for Exaone4-style GQA attention block (T=2048, HID=4096,
H=32 q-heads, HK=8 kv-heads, D=128, sliding window 1023, QK-RMSNorm + NeoX RoPE).

Sharding: tensor-parallel over heads across 8 NeuronCores. Core m owns q-heads
[4m, 4m+4) and kv-head m (GQA group-aligned), plus the matching o_proj column
slice; per-core partial outputs are summed on the host (the all-reduce).

Device design notes:
 - Attention is GQA-packed: one S^T matmul covers all 4 q-heads for a 128-query
   block (rhs columns = (head, t)), with the shared K-block / V-block as the
   stationary operand. All attention matmuls are uniform [128,128]x[128,512].
 - The per-(s-block) chain S -> mask -> exp -> PV/rowsum is emitted fine-grained
   and interleaved with qkv / o_proj projection matmuls so the PE never idles
   (keeps the HAM clock gate warm at 2.4 GHz).
 - ACT runs Exp only (one table load); QK-RMSNorm rsqrt is a Quake-style
   Newton iteration on DVE; column sums (softmax denominator, mean-square) use
   a [128,128] ones stationary operand so the result lands partition-replicated
   in PSUM - no partition broadcasts anywhere.
 - RoPE uses shared cos/sin tables plus per-partition norm-weight scalars via
   fused scalar_tensor_tensor ops; the d-half rotation is an SBUF-SBUF DMA.
 - All large matmuls use bf16 operands with fp32 PSUM accumulation.
"""

import sys

import numpy as np

if "/opt/trn_rl_repo" not in sys.path:
    sys.path.insert(0, "/opt/trn_rl_repo")

import ml_dtypes

BF16 = ml_dtypes.bfloat16

HID = 4096
H = 32
HK = 8
D = 128
WIN = 1023
THETA = 1000000.0
EPS = 1e-6
SCALE = D ** -0.5
M = 8            # cores
QH = H // M      # q heads per core (4)
NJ = QH + 2      # j-blocks in qkv^T output (4 q + 1 k + 1 v)
TB = 512         # t free-dim block
HB = 256         # half t block (x staging granularity)
NEG = -1.0e30
MAGIC = 0x5F3759DF

_PROG_CACHE = {}


def _build_program(T):
    """Build the (single-core SPMD) Bass program for sequence length T."""
    from contextlib import ExitStack

    import concourse.bass as bass  # noqa: F401
    import concourse.tile as tile
    from concourse import bacc, mybir
    from concourse.masks import make_identity

    f32 = mybir.dt.float32
    bf = mybir.dt.bfloat16
    i32 = mybir.dt.int32

    NT = T // TB          # 512-blocks (4)
    NU = T // 128         # 128-query blocks (16)
    NC = HID // 128       # contraction chunks (32)
    NOB = HID // 128      # o_proj output row blocks (32)

    mult = mybir.AluOpType.mult
    add = mybir.AluOpType.add
    sub = mybir.AluOpType.subtract
    shr = mybir.AluOpType.arith_shift_right
    Exp = mybir.ActivationFunctionType.Exp

    nc = bacc.Bacc(
        "TRN2",
        target_bir_lowering=False,
        debug=False,
        enable_asserts=False,
        num_devices=M,
    )

    # x pre-tiled on host: block (tb, cq) = [128, 4*TB] (4 c-chunks per row)
    xT_h = nc.dram_tensor(
        "xT", [NT * (HID // 512) * 128, 4 * TB], bf, kind="ExternalInput"
    )
    # qkv weights, j-major: [128, (j, c, f)]
    wT_h = nc.dram_tensor("wT", [128, NJ * NC * 128], bf, kind="ExternalInput")
    # o_proj weights, obp-major: [(obp, p), (jc, oi, o')]
    wo_h = nc.dram_tensor("woT2", [(NOB // 2) * 128, 4 * 256], bf, kind="ExternalInput")
    cos_h = nc.dram_tensor("cosT", [128, T], bf, kind="ExternalInput")
    sin_h = nc.dram_tensor("sinT", [128, T], bf, kind="ExternalInput")
    wqc_h = nc.dram_tensor("wq_c", [128, 1], f32, kind="ExternalInput")
    wqs_h = nc.dram_tensor("wq_s", [128, 1], f32, kind="ExternalInput")
    wkc_h = nc.dram_tensor("wk_c", [128, 1], f32, kind="ExternalInput")
    wks_h = nc.dram_tensor("wk_s", [128, 1], f32, kind="ExternalInput")
    maskd_h = nc.dram_tensor("maskd", [128, 512], f32, kind="ExternalInput")
    maskw_h = nc.dram_tensor("maskw", [128, 512], f32, kind="ExternalInput")
    # out pre-tiled: block (tb, obp) = [128, 2*TB]
    outT_h = nc.dram_tensor(
        "outT", [NT * (HID // 256) * 128, 2 * TB], bf, kind="ExternalOutput"
    )

    xTr = xT_h.ap().rearrange("(b p) u -> b p u", p=128)
    wTr = wT_h.ap().rearrange("p (j c f) -> p j c f", j=NJ, c=NC)
    wor = wo_h.ap().rearrange("(b p) u -> b p u", p=128)
    outr = outT_h.ap().rearrange("(b p) u -> b p u", p=128)

    with tile.TileContext(nc) as tc, ExitStack() as ctx:
        consts = ctx.enter_context(tc.tile_pool(name="consts", bufs=1))
        persist = ctx.enter_context(tc.tile_pool(name="persist", bufs=1))
        xthp = ctx.enter_context(tc.tile_pool(name="xthp", bufs=3))
        wop = ctx.enter_context(tc.tile_pool(name="wop", bufs=3))
        stp = ctx.enter_context(tc.tile_pool(name="stp", bufs=8))
        sqp = ctx.enter_context(tc.tile_pool(name="sqp", bufs=2))
        qrp = ctx.enter_context(tc.tile_pool(name="qrp", bufs=2))
        rtp = ctx.enter_context(tc.tile_pool(name="rtp", bufs=3))
        ntp = ctx.enter_context(tc.tile_pool(name="ntp", bufs=3))
        y0p = ctx.enter_context(tc.tile_pool(name="y0p", bufs=2))
        sclp = ctx.enter_context(tc.tile_pool(name="sclp", bufs=2))
        esp = ctx.enter_context(tc.tile_pool(name="esp", bufs=4))
        rbp = ctx.enter_context(tc.tile_pool(name="rbp", bufs=2))
        atp = ctx.enter_context(tc.tile_pool(name="atp", bufs=2))
        osp = ctx.enter_context(tc.tile_pool(name="osp", bufs=3))
        # PSUM: 8 banks total: 3 (S) + 1 (pv) + 2 (ms/rs) + 2 (proj chains)
        spsum = ctx.enter_context(tc.tile_pool(name="spsum", bufs=3, space="PSUM"))
        pvps = ctx.enter_context(tc.tile_pool(name="pvps", bufs=1, space="PSUM"))
        smps = ctx.enter_context(tc.tile_pool(name="smps", bufs=2, space="PSUM"))
        prps = ctx.enter_context(tc.tile_pool(name="prps", bufs=2, space="PSUM"))

        # ---- resident constants (loads emitted below, interleaved) ----------
        w_sb = consts.tile([128, NJ, NC, 128], bf)
        cos_sb = consts.tile([128, T], bf)
        sin_sb = consts.tile([128, T], bf)
        wqc_sb = consts.tile([128, 1], f32)
        wqs_sb = consts.tile([128, 1], f32)
        wkc_sb = consts.tile([128, 1], f32)
        wks_sb = consts.tile([128, 1], f32)
        maskd_sb = consts.tile([128, 512], f32)
        maskw_sb = consts.tile([128, 512], f32)
        ident = consts.tile([128, 128], bf)
        ones_bf = consts.tile([128, 128], bf)
        magic_i = consts.tile([128, TB], i32)
        one_i = consts.tile([128, TB], i32)

        # ---- persistent activations ----------------------------------------
        qT = persist.tile([128, NU, QH, 128], bf)   # roped+normed q^T
        kT = persist.tile([128, T], bf)             # roped+normed k^T
        Vt = persist.tile([128, NU, 128], bf)       # v in [s, d] layout

        attnTs = {}
        stages = {}
        xth = {}

        def emit_xth_load(tbn, h):
            """Stage x for (tb, half): [128, cq, ci, HB]."""
            t = xthp.tile([128, 8, 4, HB], bf, tag="xth", name=f"xth_{tbn}_{h}")
            for cq in range(8):
                src = xTr[tbn * 8 + cq].rearrange("p (ci u) -> p ci u", u=TB)
                nc.sync.dma_start(t[:, cq], src[:, :, h * HB : (h + 1) * HB])
            xth[(tbn, h)] = t

        def emit_qkv_chain(tbn, h, j):
            """Half-chain: qkv projection for j-block j, t columns [h*HB, h*HB+HB)."""
            ps = prps.tile([128, HB], f32, tag="proj", name=f"qkv_{tbn}_{h}_{j}")
            xt = xth[(tbn, h)]
            for cq in range(8):
                for ci in range(4):
                    c = cq * 4 + ci
                    nc.tensor.matmul(
                        ps,
                        lhsT=w_sb[:, j, c, :],
                        rhs=xt[:, cq, ci, :],
                        start=(c == 0),
                        stop=(c == NC - 1),
                    )
            if h == 0:
                stages[(tbn, j)] = stp.tile(
                    [128, TB], bf, tag="stage", name=f"st_{tbn}_{j}"
                )
            st = stages[(tbn, j)]
            nc.vector.tensor_copy(st[:, h * HB : (h + 1) * HB], ps)

        def emit_rms_rope(tbn, j):
            """RMS-normalize + RoPE j-block j of tb (j<QH: q head j; j==QH: k)."""
            t0 = tbn * TB
            ts_ = slice(t0, t0 + TB)
            st = stages.pop((tbn, j))
            # d-half rotation via SBUF->SBUF DMA (runs while rms computes)
            qr = qrp.tile([128, TB], bf, tag="qrot", name=f"qr_{tbn}_{j}")
            nc.gpsimd.dma_start(qr[0:64, :], st[64:128, :])
            nc.gpsimd.dma_start(qr[64:128, :], st[0:64, :])
            # mean-square via ones-matmul (partition-replicated result)
            sq = sqp.tile([128, TB], bf, tag="sq", name=f"sq_{tbn}_{j}")
            nc.vector.tensor_tensor(sq, st, st, mult)
            ms = smps.tile([128, TB], f32, tag="small", name=f"ms_{tbn}_{j}")
            nc.tensor.matmul(ms, lhsT=ones_bf, rhs=sq, start=True, stop=True)
            msf = ntp.tile([128, TB], f32, tag="nt", name=f"msf_{tbn}_{j}")
            nc.vector.tensor_copy(msf, ms)
            # rsqrt(ms) via magic-constant seed + 1 Newton step (all DVE).
            # sqrt(D) and the 1/sqrt(D) score scale are folded into the host
            # tables, so the raw column sum-of-squares is the right input.
            sh = ntp.tile([128, TB], i32, tag="nt", name=f"sh_{tbn}_{j}")
            nc.vector.tensor_tensor(sh, msf.bitcast(i32), one_i, shr)
            y0i = y0p.tile([128, TB], i32, tag="y0", name=f"y0_{tbn}_{j}")
            nc.vector.tensor_tensor(y0i, magic_i, sh, sub)
            y0 = y0i.bitcast(f32)
            a2 = ntp.tile([128, TB], f32, tag="nt", name=f"a2_{tbn}_{j}")
            nc.vector.tensor_tensor(a2, y0, y0, mult)
            b2 = ntp.tile([128, TB], f32, tag="nt", name=f"b2_{tbn}_{j}")
            nc.vector.tensor_tensor(b2, a2, msf, mult)
            c2 = ntp.tile([128, TB], f32, tag="nt", name=f"c2_{tbn}_{j}")
            nc.vector.tensor_scalar(c2, b2, -0.5, 1.5, mult, add)
            scl = sclp.tile([128, TB], f32, tag="scl", name=f"scl_{tbn}_{j}")
            nc.vector.tensor_tensor(scl, y0, c2, mult)
            # rope: dest = (st*w_c*cos + rot(st)*w_s*sin) * scl
            wc, ws = (wqc_sb, wqs_sb) if j < QH else (wkc_sb, wks_sb)
            a = rtp.tile([128, TB], f32, tag="rt", name=f"ra_{tbn}_{j}")
            nc.vector.scalar_tensor_tensor(a, st, wc, cos_sb[:, ts_], mult, mult)
            b = rtp.tile([128, TB], f32, tag="rt", name=f"rb_{tbn}_{j}")
            nc.vector.scalar_tensor_tensor(b, qr, ws, sin_sb[:, ts_], mult, mult)
            cc = rtp.tile([128, TB], f32, tag="rt", name=f"rc_{tbn}_{j}")
            nc.vector.tensor_tensor(cc, a, b, add)
            if j < QH:
                dest = qT[:, 4 * tbn : 4 * tbn + 4, j, :]
            else:
                dest = kT[:, ts_]
            nc.vector.tensor_tensor(dest, cc, scl, mult)

        def emit_vtrans(tbn):
            """v: transpose [d, t] -> [s, d] blocks via PE."""
            st = stages.pop((tbn, NJ - 1))
            for q in range(4):
                pst = prps.tile([128, 128], bf, tag="proj", name=f"vt_{tbn}_{q}")
                nc.tensor.transpose(pst, st[:, q * 128 : (q + 1) * 128], ident)
                nc.vector.tensor_copy(Vt[:, tbn * 4 + q, :], pst)

        def emit_attn(u):
            """Attention for query block u: all 4 heads packed per matmul."""
            first = max(0, u - 8)
            sbs = list(range(first, u + 1))
            tbn = u // 4
            ur = u % 4
            if ur == 0:
                attnTs[tbn] = atp.tile(
                    [128, QH, TB], bf, tag="attnT", name=f"attnT_{tbn}"
                )
            pv = pvps.tile([128, TB], f32, tag="pv", name=f"pv_{u}")
            rs = smps.tile([128, TB], f32, tag="small", name=f"rs_{u}")
            for i, sb in enumerate(sbs):
                ps = spsum.tile([128, TB], f32, tag="spsum", name=f"s_{u}_{sb}")
                nc.tensor.matmul(
                    ps,
                    lhsT=kT[:, sb * 128 : (sb + 1) * 128],
                    rhs=qT[:, u],
                    start=True,
                    stop=True,
                )
                if sb == u:
                    nc.vector.tensor_tensor(ps, ps, maskd_sb, add)
                elif u - sb == 8:
                    nc.vector.tensor_tensor(ps, ps, maskw_sb, add)
                es = esp.tile([128, TB], bf, tag="es", name=f"es_{u}_{sb}")
                nc.scalar.activation(es, ps, Exp)
                last = i == len(sbs) - 1
                nc.tensor.matmul(
                    pv, lhsT=Vt[:, sb, :], rhs=es,
                    start=(i == 0), stop=last, skip_group_check=True,
                )
                nc.tensor.matmul(
                    rs, lhsT=ones_bf, rhs=es,
                    start=(i == 0), stop=last, skip_group_check=True,
                )
            rb = rbp.tile([128, TB], f32, tag="rbn", name=f"rbn_{u}")
            nc.vector.reciprocal_approx_fast(rb, rs)
            at = attnTs[tbn]
            nc.vector.tensor_tensor(
                at[:, :, ur * 128 : (ur + 1) * 128],
                pv.rearrange("p (h t) -> p h t", t=128),
                rb.rearrange("p (h t) -> p h t", t=128),
                mult,
            )

        def emit_oproj(tbn, obp):
            """o_proj for (tb, ob-pair): wo streamed from DRAM per chunk."""
            at = attnTs[tbn]
            wot = wop.tile([128, 4, 2, 128], bf, tag="wo", name=f"wo_{tbn}_{obp}")
            nc.sync.dma_start(
                wot, wor[obp].rearrange("p (jc oi o) -> p jc oi o", jc=4, oi=2)
            )
            ost = osp.tile([128, 2, TB], bf, tag="ost", name=f"ost_{tbn}_{obp}")
            for oi in range(2):
                ps = prps.tile(
                    [128, TB], f32, tag="proj", name=f"op_{tbn}_{obp}_{oi}"
                )
                for jc in range(QH):
                    nc.tensor.matmul(
                        ps,
                        lhsT=wot[:, jc, oi, :],
                        rhs=at[:, jc, :],
                        start=(jc == 0),
                        stop=(jc == QH - 1),
                    )
                nc.vector.tensor_copy(ost[:, oi, :], ps)
            nc.gpsimd.dma_start(
                outr[tbn * (NOB // 2) + obp].rearrange("p (oi u) -> p oi u", u=TB),
                ost,
            )

        # ---- prologue -------------------------------------------------------
        # Interleave the first x/w DMAs so matmuls start as soon as chunk 0
        # of x and j-block 0 of w have landed.
        emit_xth_load(0, 0)
        nc.sync.dma_start(w_sb[:, 0], wTr[:, 0])
        nc.sync.dma_start(w_sb[:, 1], wTr[:, 1])
        emit_xth_load(0, 1)
        for j in range(2, NJ):
            nc.sync.dma_start(w_sb[:, j], wTr[:, j])
        nc.sync.dma_start(cos_sb, cos_h.ap())
        nc.sync.dma_start(sin_sb, sin_h.ap())
        for t_, h_ in (
            (wqc_sb, wqc_h), (wqs_sb, wqs_h), (wkc_sb, wkc_h), (wks_sb, wks_h),
            (maskd_sb, maskd_h), (maskw_sb, maskw_h),
        ):
            nc.sync.dma_start(t_, h_.ap())
        make_identity(nc, ident)
        nc.vector.memset(ones_bf, 1.0)
        nc.gpsimd.memset(magic_i, MAGIC)
        nc.gpsimd.memset(one_i, 1)

        for h in range(2):
            for j in range(NJ):
                emit_qkv_chain(0, h, j)
                if h == 1:
                    if j < NJ - 1:
                        emit_rms_rope(0, j)
                    else:
                        emit_vtrans(0)
        emit_xth_load(1, 0)
        emit_xth_load(1, 1)

        # chains of tb+1 emitted during tb: (half, j) per quarter-block step
        CHAIN_SCHED = [
            [(0, 0), (0, 1), (0, 2)],
            [(0, 3), (0, 4), (0, 5)],
            [(1, 0), (1, 1), (1, 2)],
            [(1, 3), (1, 4), (1, 5)],
        ]

        # ---- steady state ---------------------------------------------------
        for tbn in range(NT):
            for ur in range(4):
                u = 4 * tbn + ur
                emit_attn(u)
                if tbn >= 1:
                    for obp in range(ur * 4, ur * 4 + 4):
                        emit_oproj(tbn - 1, obp)
                if tbn + 1 < NT:
                    for (h, j) in CHAIN_SCHED[ur]:
                        emit_qkv_chain(tbn + 1, h, j)
                        if h == 1:
                            if j < NJ - 1:
                                emit_rms_rope(tbn + 1, j)
                            else:
                                emit_vtrans(tbn + 1)
                if tbn + 2 < NT:
                    if ur == 0:
                        emit_xth_load(tbn + 2, 0)
                    elif ur == 2:
                        emit_xth_load(tbn + 2, 1)
        # ---- epilogue -------------------------------------------------------
        for obp in range(NOB // 2):
            emit_oproj(NT - 1, obp)

    nc.compile()
    return nc


def _get_program(T):
    if T not in _PROG_CACHE:
        _PROG_CACHE[T] = _build_program(T)
    return _PROG_CACHE[T]


def _host_prep(positions, hidden_states, wqkv, wo, q_norm_w, k_norm_w):
    """Build the 8 per-core input maps (host-side sharding + table prep)."""
    T = hidden_states.shape[0]
    pos = np.asarray(positions).astype(np.float64)
    hs = np.asarray(hidden_states, dtype=np.float32)
    wqkv = np.asarray(wqkv, dtype=np.float32)
    wo = np.asarray(wo, dtype=np.float32)
    qw = np.asarray(q_norm_w, dtype=np.float64)
    kw = np.asarray(k_norm_w, dtype=np.float64)

    half = D // 2
    inv_freq = 1.0 / (THETA ** (np.arange(0, D, 2, dtype=np.float64) / D))  # [64]
    th = pos[:, None] * inv_freq[None, :]          # [T, 64]
    cos = np.cos(th).T                             # [64, T]
    sin = np.sin(th).T

    # shared rope tables: rows (cos||cos), (-sin||+sin)
    cosT = np.concatenate([cos, cos], axis=0).astype(BF16)          # [128, T]
    sinT = np.concatenate([-sin, sin], axis=0).astype(BF16)

    # per-partition norm-weight scalars; q side folds SCALE*sqrt(D)=1, k side
    # folds sqrt(D) (the device rsqrt is of the raw sum of squares).
    sqD = float(np.sqrt(D))
    wq_c = qw.astype(np.float32).reshape(128, 1)
    wq_s = np.concatenate([qw[half:], qw[:half]]).astype(np.float32).reshape(128, 1)
    wk_c = (kw * sqD).astype(np.float32).reshape(128, 1)
    wk_s = (np.concatenate([kw[half:], kw[:half]]) * sqD).astype(np.float32).reshape(128, 1)

    # masks over (s in 128, (h in 4) x (t in 128))
    si = np.arange(128)[:, None]
    ti = np.arange(128)[None, :]
    md = np.where(ti >= si, 0.0, NEG).astype(np.float32)   # diag: keep t >= s
    mw = np.where(ti < si, 0.0, NEG).astype(np.float32)    # window edge: t < s
    maskd = np.tile(md, (1, 4))
    maskw = np.tile(mw, (1, 4))

    # x tiled: block (tb, cq) = [128, 4*TB]
    NTb, NCq = T // TB, HID // 512
    xT = np.ascontiguousarray(
        hs.T.reshape(NCq, 4, 128, NTb, TB)
        .transpose(3, 0, 2, 1, 4)
        .reshape(NTb * NCq * 128, 4 * TB)
    ).astype(BF16)

    in_maps = []
    for m in range(M):
        wq_m = wqkv[m * QH * D : (m + 1) * QH * D]            # [512, HID]
        wk_m = wqkv[H * D + m * D : H * D + (m + 1) * D]      # [128, HID]
        wv_m = wqkv[(H + HK) * D + m * D : (H + HK) * D + (m + 1) * D]
        Wm = np.concatenate([wq_m, wk_m, wv_m], axis=0)       # [768, HID]
        # [p, j, c, f] layout
        wT = np.ascontiguousarray(
            Wm.T.reshape(HID // 128, 128, NJ, 128)
            .transpose(1, 2, 0, 3)
            .reshape(128, NJ * (HID // 128) * 128)
        ).astype(BF16)
        # o_proj: [obp, p, jc, oi, o'] -> [(obp p), (jc oi o')]
        wom = wo[:, m * QH * D : (m + 1) * QH * D]            # [HID, 512]
        woT2 = np.ascontiguousarray(
            wom.reshape(HID // 256, 2, 128, 4, 128)
            .transpose(0, 4, 3, 1, 2)
            .reshape((HID // 256) * 128, 4 * 256)
        ).astype(BF16)
        in_maps.append(
            {
                "xT": xT,
                "wT": wT,
                "woT2": woT2,
                "cosT": cosT,
                "sinT": sinT,
                "wq_c": wq_c,
                "wq_s": wq_s,
                "wk_c": wk_c,
                "wk_s": wk_s,
                "maskd": maskd,
                "maskw": maskw,
            }
        )
    return in_maps


def _run(in_maps, T, trace=False):
    from concourse import bass_utils

    nc = _get_program(T)
    res = bass_utils.run_bass_kernel_spmd(
        nc, in_maps, core_ids=list(range(M)), trace=trace
    )
    return res


def kernel(positions, hidden_states, wqkv, wo, q_norm_w, k_norm_w, _trace=False):
    T = hidden_states.shape[0]
    in_maps = _host_prep(positions, hidden_states, wqkv, wo, q_norm_w, k_norm_w)
    res = _run(in_maps, T, trace=_trace)
    NTb, NOBp = T // TB, HID // 256
    acc = np.zeros((NTb, NOBp, 128, 2, TB), np.float64)
    for r in res.results:
        acc += r["outT"].astype(np.float64).reshape(NTb, NOBp, 128, 2, TB)
    # untile: out[t, o] with o = (2*obp + oi)*128 + p, t = tb*TB + u
    out = np.ascontiguousarray(
        acc.transpose(0, 4, 1, 3, 2).reshape(T, HID)
    ).astype(np.float32)
    kernel._last_results = res
    return out


# revision 24
# speedup vs baseline: 1.2077x; 1.1008x over previous
"""Trainium2 Bass kernel for Exaone4-style GQA attention block (T=2048, HID=4096,
H=32 q-heads, HK=8 kv-heads, D=128, sliding window 1023, QK-RMSNorm + NeoX RoPE).

Sharding: tensor-parallel over heads across 8 NeuronCores. Core m owns q-heads
[4m, 4m+4) and kv-head m (GQA group-aligned), plus the matching o_proj column
slice; per-core partial outputs are summed on the host (the all-reduce).

Device design notes:
 - Attention is GQA-packed: one S^T matmul covers all 4 q-heads for a 128-query
   block (rhs columns = (head, t)), with the shared K-block / V-block as the
   stationary operand. All attention matmuls are uniform [128,128]x[128,512].
 - The per-(s-block) chain S -> mask -> exp -> PV/rowsum is emitted fine-grained
   and interleaved with qkv / o_proj projection matmuls so the PE never idles
   (keeps the HAM clock gate warm at 2.4 GHz).
 - ACT runs Exp only (one table load); QK-RMSNorm rsqrt is a Quake-style
   Newton iteration on DVE; column sums (softmax denominator, mean-square) use
   a [128,128] ones stationary operand so the result lands partition-replicated
   in PSUM - no partition broadcasts anywhere.
 - RoPE uses shared cos/sin tables plus per-partition norm-weight scalars via
   fused scalar_tensor_tensor ops; the d-half rotation is an SBUF-SBUF DMA.
 - All large matmuls use bf16 operands with fp32 PSUM accumulation.
"""

import sys

import numpy as np

if "/opt/trn_rl_repo" not in sys.path:
    sys.path.insert(0, "/opt/trn_rl_repo")

import ml_dtypes

BF16 = ml_dtypes.bfloat16

HID = 4096
H = 32
HK = 8
D = 128
WIN = 1023
THETA = 1000000.0
EPS = 1e-6
SCALE = D ** -0.5
M = 8            # cores
QH = H // M      # q heads per core (4)
NJ = QH + 2      # j-blocks in qkv^T output (4 q + 1 k + 1 v)
TB = 512         # t free-dim block
HB = 256         # half t block (x staging granularity)
NEG = -1.0e30
MAGIC = 0x5F3759DF

_PROG_CACHE = {}


def _build_program(T):
    """Build the (single-core SPMD) Bass program for sequence length T."""
    from contextlib import ExitStack

    import concourse.bass as bass  # noqa: F401
    import concourse.tile as tile
    from concourse import bacc, mybir
    from concourse.masks import make_identity

    f32 = mybir.dt.float32
    bf = mybir.dt.bfloat16
    i32 = mybir.dt.int32

    NT = T // TB          # 512-blocks (4)
    NU = T // 128         # 128-query blocks (16)
    NC = HID // 128       # contraction chunks (32)
    NOB = HID // 128      # o_proj output row blocks (32)

    mult = mybir.AluOpType.mult
    add = mybir.AluOpType.add
    sub = mybir.AluOpType.subtract
    shr = mybir.AluOpType.arith_shift_right
    Exp = mybir.ActivationFunctionType.Exp

    nc = bacc.Bacc(
        "TRN2",
        target_bir_lowering=False,
        debug=False,
        enable_asserts=False,
        num_devices=M,
    )

    # x pre-tiled on host: block (tb, half) = [128, (cq ci u)] fully contiguous
    xT_h = nc.dram_tensor(
        "xT", [NT * 2 * 128, (HID // 128) * HB], bf, kind="ExternalInput"
    )
    # qkv weights, j-major: [128, (j, c, f)]
    wT_h = nc.dram_tensor("wT", [128, NJ * NC * 128], bf, kind="ExternalInput")
    # o_proj weights, obp-major: [(obp, p), (jc, oi, o')]
    wo_h = nc.dram_tensor("woT2", [(NOB // 2) * 128, 4 * 256], bf, kind="ExternalInput")
    cwq_h = nc.dram_tensor("cwq", [128, T], bf, kind="ExternalInput")
    swq_h = nc.dram_tensor("swq", [128, T], bf, kind="ExternalInput")
    cwk_h = nc.dram_tensor("cwk", [128, T], bf, kind="ExternalInput")
    swk_h = nc.dram_tensor("swk", [128, T], bf, kind="ExternalInput")
    maskd_h = nc.dram_tensor("maskd", [128, 512], bf, kind="ExternalInput")
    maskw_h = nc.dram_tensor("maskw", [128, 512], bf, kind="ExternalInput")
    # out pre-tiled: block (tb, obp) = [128, 2*TB]
    outT_h = nc.dram_tensor(
        "outT", [NT * (HID // 256) * 128, 2 * TB], bf, kind="ExternalOutput"
    )

    wTr = wT_h.ap().rearrange("p (j c f) -> p j c f", j=NJ, c=NC)

    with tile.TileContext(nc) as tc, ExitStack() as ctx:
        consts = ctx.enter_context(tc.tile_pool(name="consts", bufs=1))
        persist = ctx.enter_context(tc.tile_pool(name="persist", bufs=1))
        xthp = ctx.enter_context(tc.tile_pool(name="xthp", bufs=3))
        wop = ctx.enter_context(tc.tile_pool(name="wop", bufs=3))
        stp = ctx.enter_context(tc.tile_pool(name="stp", bufs=4))
        sqp = ctx.enter_context(tc.tile_pool(name="sqp", bufs=2))
        qrp = ctx.enter_context(tc.tile_pool(name="qrp", bufs=2))
        rtp = ctx.enter_context(tc.tile_pool(name="rtp", bufs=3))
        ntp = ctx.enter_context(tc.tile_pool(name="ntp", bufs=2))
        y0p = ctx.enter_context(tc.tile_pool(name="y0p", bufs=2))
        sclp = ctx.enter_context(tc.tile_pool(name="sclp", bufs=2))
        esp = ctx.enter_context(tc.tile_pool(name="esp", bufs=4))
        rbp = ctx.enter_context(tc.tile_pool(name="rbp", bufs=2))
        atp = ctx.enter_context(tc.tile_pool(name="atp", bufs=2))
        osp = ctx.enter_context(tc.tile_pool(name="osp", bufs=2))
        # PSUM: 8 banks total: 3 (S) + 1 (pv) + 2 (ms/rs) + 2 (proj chains)
        spsum = ctx.enter_context(tc.tile_pool(name="spsum", bufs=3, space="PSUM"))
        pvps = ctx.enter_context(tc.tile_pool(name="pvps", bufs=1, space="PSUM"))
        smps = ctx.enter_context(tc.tile_pool(name="smps", bufs=2, space="PSUM"))
        prps = ctx.enter_context(tc.tile_pool(name="prps", bufs=2, space="PSUM"))

        # ---- resident constants (loads emitted below, interleaved) ----------
        w_sb = consts.tile([128, NJ, NC, 128], bf)
        cwq_sb = consts.tile([128, T], bf)
        swq_sb = consts.tile([128, T], bf)
        cwk_sb = consts.tile([128, T], bf)
        swk_sb = consts.tile([128, T], bf)
        maskd_sb = consts.tile([128, 512], bf)
        maskw_sb = consts.tile([128, 512], bf)
        ident = consts.tile([128, 128], bf)
        ones_bf = consts.tile([128, 128], bf)
        magic_i = consts.tile([128, TB], i32)
        one_i = consts.tile([128, TB], i32)

        # ---- persistent activations ----------------------------------------
        qT = persist.tile([128, NU, QH, 128], bf)   # roped+normed q^T
        kT = persist.tile([128, T], bf)             # roped+normed k^T
        Vt = persist.tile([128, NU, 128], bf)       # v in [s, d] layout

        attnTs = {}
        stages = {}
        xth = {}

        xTr = xT_h.ap().rearrange("(b p) u -> b p u", p=128)

        def emit_xth_load(tbn, h):
            """Stage x for (tb, half): [128, cq, ci, HB] in one DMA."""
            t = xthp.tile([128, 8, 4, HB], bf, tag="xth", name=f"xth_{tbn}_{h}")
            nc.sync.dma_start(
                t,
                xTr[tbn * 2 + h].rearrange("p (cq ci u) -> p cq ci u", cq=8, ci=4),
            )
            xth[(tbn, h)] = t

        def emit_qkv_chain(tbn, h, j):
            """Half-chain: qkv projection for j-block j, t columns [h*HB, h*HB+HB)."""
            ps = prps.tile([128, HB], f32, tag="proj", name=f"qkv_{tbn}_{h}_{j}")
            xt = xth[(tbn, h)]
            for cq in range(8):
                for ci in range(4):
                    c = cq * 4 + ci
                    nc.tensor.matmul(
                        ps,
                        lhsT=w_sb[:, j, c, :],
                        rhs=xt[:, cq, ci, :],
                        start=(c == 0),
                        stop=(c == NC - 1),
                    )
            if h == 0:
                stages[(tbn, j)] = stp.tile(
                    [128, TB], bf, tag="stage", name=f"st_{tbn}_{j}"
                )
            st = stages[(tbn, j)]
            nc.vector.tensor_copy(st[:, h * HB : (h + 1) * HB], ps)

        def emit_rms_rope(tbn, j):
            """RMS-normalize + RoPE j-block j of tb (j<QH: q head j; j==QH: k)."""
            t0 = tbn * TB
            ts_ = slice(t0, t0 + TB)
            st = stages.pop((tbn, j))
            # d-half rotation via SBUF->SBUF DMA (runs while rms computes)
            qr = qrp.tile([128, TB], bf, tag="qrot", name=f"qr_{tbn}_{j}")
            nc.gpsimd.dma_start(qr[0:64, :], st[64:128, :])
            nc.gpsimd.dma_start(qr[64:128, :], st[0:64, :])
            # mean-square via ones-matmul (partition-replicated result)
            sq = sqp.tile([128, TB], bf, tag="sq", name=f"sq_{tbn}_{j}")
            nc.vector.tensor_tensor(sq, st, st, mult)
            ms = smps.tile([128, TB], f32, tag="small", name=f"ms_{tbn}_{j}")
            nc.tensor.matmul(ms, lhsT=ones_bf, rhs=sq, start=True, stop=True)
            # rsqrt(ms) via magic-constant seed + 1 Newton step (all DVE).
            # sqrt(D) and the 1/sqrt(D) score scale are folded into the host
            # tables, so the raw column sum-of-squares is the right input.
            sh = ntp.tile([128, TB], i32, tag="nt", name=f"sh_{tbn}_{j}")
            nc.vector.tensor_tensor(sh, ms.bitcast(i32), one_i, shr)
            y0i = y0p.tile([128, TB], i32, tag="y0", name=f"y0_{tbn}_{j}")
            nc.vector.tensor_tensor(y0i, magic_i, sh, sub)
            y0 = y0i.bitcast(f32)
            a2 = ntp.tile([128, TB], f32, tag="nt", name=f"a2_{tbn}_{j}")
            nc.vector.tensor_tensor(a2, y0, y0, mult)
            d2 = ntp.tile([128, TB], f32, tag="nt", name=f"d2_{tbn}_{j}")
            nc.vector.scalar_tensor_tensor(d2, a2, -0.5, ms, mult, mult)
            scl = sclp.tile([128, TB], f32, tag="scl", name=f"scl_{tbn}_{j}")
            nc.vector.scalar_tensor_tensor(scl, d2, 1.5, y0, add, mult)
            # rope: dest = (st*cw + rot(st)*sw) * scl; the norm weights and
            # score scale are folded into the host tables. The two table
            # multiplies run on GpSimd (plain TT) to unload the DVE.
            cw, sw = (cwq_sb, swq_sb) if j < QH else (cwk_sb, swk_sb)
            a = rtp.tile([128, TB], f32, tag="rt", name=f"ra_{tbn}_{j}")
            nc.gpsimd.tensor_tensor(a, st, cw[:, ts_], mult)
            b = rtp.tile([128, TB], f32, tag="rt", name=f"rb_{tbn}_{j}")
            nc.gpsimd.tensor_tensor(b, qr, sw[:, ts_], mult)
            cc = rtp.tile([128, TB], f32, tag="rt", name=f"rc_{tbn}_{j}")
            nc.vector.tensor_tensor(cc, a, b, add)
            if j < QH:
                dest = qT[:, 4 * tbn : 4 * tbn + 4, j, :]
            else:
                dest = kT[:, ts_]
            nc.vector.tensor_tensor(dest, cc, scl, mult)

        def emit_vtrans(tbn):
            """v: transpose [d, t] -> [s, d] blocks via PE."""
            st = stages.pop((tbn, NJ - 1))
            for q in range(4):
                pst = prps.tile([128, 128], bf, tag="proj", name=f"vt_{tbn}_{q}")
                nc.tensor.transpose(pst, st[:, q * 128 : (q + 1) * 128], ident)
                nc.vector.tensor_copy(Vt[:, tbn * 4 + q, :], pst)

        def emit_attn(u):
            """Attention for query block u: all 4 heads packed per matmul."""
            first = max(0, u - 8)
            sbs = list(range(first, u + 1))
            tbn = u // 4
            ur = u % 4
            if ur == 0:
                attnTs[tbn] = atp.tile(
                    [128, QH, TB], bf, tag="attnT", name=f"attnT_{tbn}"
                )
            pv = pvps.tile([128, TB], f32, tag="pv", name=f"pv_{u}")
            rs = smps.tile([128, TB], f32, tag="small", name=f"rs_{u}")
            for i, sb in enumerate(sbs):
                ps = spsum.tile([128, TB], f32, tag="spsum", name=f"s_{u}_{sb}")
                nc.tensor.matmul(
                    ps,
                    lhsT=kT[:, sb * 128 : (sb + 1) * 128],
                    rhs=qT[:, u],
                    start=True,
                    stop=True,
                )
                if sb == u:
                    nc.vector.tensor_tensor(ps, ps, maskd_sb, add)
                elif u - sb == 8:
                    nc.vector.tensor_tensor(ps, ps, maskw_sb, add)
                es = esp.tile([128, TB], bf, tag="es", name=f"es_{u}_{sb}")
                nc.scalar.activation(es, ps, Exp)
                last = i == len(sbs) - 1
                nc.tensor.matmul(
                    pv, lhsT=Vt[:, sb, :], rhs=es,
                    start=(i == 0), stop=last, skip_group_check=True,
                )
                nc.tensor.matmul(
                    rs, lhsT=ones_bf, rhs=es,
                    start=(i == 0), stop=last, skip_group_check=True,
                )
            rb = rbp.tile([128, TB], f32, tag="rbn", name=f"rbn_{u}")
            nc.vector.reciprocal_approx_fast(rb, rs)
            at = attnTs[tbn]
            nc.vector.tensor_tensor(
                at[:, :, ur * 128 : (ur + 1) * 128],
                pv.rearrange("p (h t) -> p h t", t=128),
                rb.rearrange("p (h t) -> p h t", t=128),
                mult,
            )

        worP = wo_h.ap().rearrange(
            "(b p) (jc oi o) -> p b jc oi o", p=128, jc=4, oi=2
        )
        outP = outT_h.ap().rearrange("(b p) (oi u) -> p b oi u", p=128, u=TB)

        def emit_oproj_pair(tbn, pp):
            """o_proj for (tb, obp-pair pp): wo streamed, one out DMA."""
            at = attnTs[tbn]
            wot = wop.tile([128, 2, 4, 2, 128], bf, tag="wo", name=f"wo_{tbn}_{pp}")
            nc.sync.dma_start(wot, worP[:, 2 * pp : 2 * pp + 2])
            ost = osp.tile([128, 2, 2, TB], bf, tag="ost", name=f"ost_{tbn}_{pp}")
            for bi in range(2):
                for oi in range(2):
                    ps = prps.tile(
                        [128, TB], f32, tag="proj", name=f"op_{tbn}_{pp}_{bi}_{oi}"
                    )
                    for jc in range(QH):
                        nc.tensor.matmul(
                            ps,
                            lhsT=wot[:, bi, jc, oi, :],
                            rhs=at[:, jc, :],
                            start=(jc == 0),
                            stop=(jc == QH - 1),
                        )
                    nc.vector.tensor_copy(ost[:, bi, oi, :], ps)
            base = tbn * (NOB // 2) + 2 * pp
            nc.gpsimd.dma_start(outP[:, base : base + 2], ost)

        # ---- prologue -------------------------------------------------------
        # Interleave the first x/w DMAs so matmuls start as soon as the first
        # half of x and the first j-block of w have landed.
        PROLOG_J = [4, 5, 0, 1, 2, 3]
        emit_xth_load(0, 0)
        nc.sync.dma_start(w_sb[:, 4], wTr[:, 4])
        nc.sync.dma_start(w_sb[:, 5], wTr[:, 5])
        emit_xth_load(0, 1)
        for j in (0, 1, 2, 3):
            nc.sync.dma_start(w_sb[:, j], wTr[:, j])
        for t_, h_ in (
            (cwq_sb, cwq_h), (swq_sb, swq_h), (cwk_sb, cwk_h), (swk_sb, swk_h),
            (maskd_sb, maskd_h), (maskw_sb, maskw_h),
        ):
            nc.sync.dma_start(t_, h_.ap())
        make_identity(nc, ident)
        nc.vector.memset(ones_bf, 1.0)
        nc.gpsimd.memset(magic_i, MAGIC)
        nc.gpsimd.memset(one_i, 1)

        for j in PROLOG_J:
            emit_qkv_chain(0, 0, j)
            emit_qkv_chain(0, 1, j)
            if j < NJ - 1:
                emit_rms_rope(0, j)
            else:
                emit_vtrans(0)
        emit_xth_load(1, 0)
        emit_xth_load(1, 1)

        # chains of tb+1 emitted during tb over ur0..ur2 (k and v first so the
        # shared kT/Vt ropes land with maximal slack; ur3 stays chain-free so
        # the last q rope finishes well before tb+1's first S matmul)
        CHAIN_SCHED = [
            [(0, 4), (1, 4), (0, 5), (1, 5)],
            [(0, 0), (1, 0), (0, 1), (1, 1)],
            [(0, 2), (1, 2), (0, 3), (1, 3)],
            [],
        ]
        # o_proj pairs per ur: more at ur3 to fill the chain-free stretch
        OPROJ_SCHED = [(0, 1), (1, 3), (3, 5), (5, 8)]

        # ---- steady state ---------------------------------------------------
        for tbn in range(NT):
            for ur in range(4):
                u = 4 * tbn + ur
                emit_attn(u)
                if tbn >= 1:
                    for pp in range(*OPROJ_SCHED[ur]):
                        emit_oproj_pair(tbn - 1, pp)
                if tbn + 1 < NT:
                    for (h, j) in CHAIN_SCHED[ur]:
                        emit_qkv_chain(tbn + 1, h, j)
                        if h == 1:
                            if j < NJ - 1:
                                emit_rms_rope(tbn + 1, j)
                            else:
                                emit_vtrans(tbn + 1)
                if tbn + 2 < NT and ur == 3:
                    emit_xth_load(tbn + 2, 0)
                    emit_xth_load(tbn + 2, 1)
        # ---- epilogue -------------------------------------------------------
        for pp in range(NOB // 4):
            emit_oproj_pair(NT - 1, pp)

    nc.compile()
    return nc


def _get_program(T):
    if T not in _PROG_CACHE:
        _PROG_CACHE[T] = _build_program(T)
    return _PROG_CACHE[T]


def _host_prep(positions, hidden_states, wqkv, wo, q_norm_w, k_norm_w):
    """Build the 8 per-core input maps (host-side sharding + table prep)."""
    T = hidden_states.shape[0]
    pos = np.asarray(positions).astype(np.float64)
    hs = np.asarray(hidden_states, dtype=np.float32)
    wqkv = np.asarray(wqkv, dtype=np.float32)
    wo = np.asarray(wo, dtype=np.float32)
    qw = np.asarray(q_norm_w, dtype=np.float64)
    kw = np.asarray(k_norm_w, dtype=np.float64)

    half = D // 2
    inv_freq = 1.0 / (THETA ** (np.arange(0, D, 2, dtype=np.float64) / D))  # [64]
    th = pos[:, None] * inv_freq[None, :]          # [T, 64]
    cos = np.cos(th).T                             # [64, T]
    sin = np.sin(th).T

    # rope tables with norm weights folded in; q side folds SCALE*sqrt(D)=1,
    # k side folds sqrt(D) (the device rsqrt is of the raw sum of squares).
    sqD = float(np.sqrt(D))

    def tables(w, scale):
        cw = np.empty((D, T), np.float64)
        sw = np.empty((D, T), np.float64)
        cw[:half] = cos * (w[:half, None] * scale)
        cw[half:] = cos * (w[half:, None] * scale)
        sw[:half] = -sin * (w[half:, None] * scale)
        sw[half:] = sin * (w[:half, None] * scale)
        return cw.astype(BF16), sw.astype(BF16)

    cwq, swq = tables(qw, 1.0)
    cwk, swk = tables(kw, sqD)

    # masks over (s in 128, (h in 4) x (t in 128))
    si = np.arange(128)[:, None]
    ti = np.arange(128)[None, :]
    md = np.where(ti >= si, 0.0, NEG).astype(BF16)   # diag: keep t >= s
    mw = np.where(ti < si, 0.0, NEG).astype(BF16)    # window edge: t < s
    maskd = np.tile(md, (1, 4))
    maskw = np.tile(mw, (1, 4))

    # x tiled: block (tb, half) = [128, (cq ci u)] contiguous
    NTb, NCq = T // TB, HID // 512
    xT = np.ascontiguousarray(
        hs.T.reshape(NCq, 4, 128, NTb, 2, HB)
        .transpose(3, 4, 2, 0, 1, 5)
        .reshape(NTb * 2 * 128, NCq * 4 * HB)
    ).astype(BF16)

    in_maps = []
    for m in range(M):
        wq_m = wqkv[m * QH * D : (m + 1) * QH * D]            # [512, HID]
        wk_m = wqkv[H * D + m * D : H * D + (m + 1) * D]      # [128, HID]
        wv_m = wqkv[(H + HK) * D + m * D : (H + HK) * D + (m + 1) * D]
        Wm = np.concatenate([wq_m, wk_m, wv_m], axis=0)       # [768, HID]
        # [p, j, c, f] layout
        wT = np.ascontiguousarray(
            Wm.T.reshape(HID // 128, 128, NJ, 128)
            .transpose(1, 2, 0, 3)
            .reshape(128, NJ * (HID // 128) * 128)
        ).astype(BF16)
        # o_proj: [obp, p, jc, oi, o'] -> [(obp p), (jc oi o')]
        wom = wo[:, m * QH * D : (m + 1) * QH * D]            # [HID, 512]
        woT2 = np.ascontiguousarray(
            wom.reshape(HID // 256, 2, 128, 4, 128)
            .transpose(0, 4, 3, 1, 2)
            .reshape((HID // 256) * 128, 4 * 256)
        ).astype(BF16)
        in_maps.append(
            {
                "xT": xT,
                "wT": wT,
                "woT2": woT2,
                "cwq": cwq,
                "swq": swq,
                "cwk": cwk,
                "swk": swk,
                "maskd": maskd,
                "maskw": maskw,
            }
        )
    return in_maps


def _run(in_maps, T, trace=False):
    from concourse import bass_utils

    nc = _get_program(T)
    res = bass_utils.run_bass_kernel_spmd(
        nc, in_maps, core_ids=list(range(M)), trace=trace
    )
    return res


def kernel(positions, hidden_states, wqkv, wo, q_norm_w, k_norm_w, _trace=False):
    T = hidden_states.shape[0]
    in_maps = _host_prep(positions, hidden_states, wqkv, wo, q_norm_w, k_norm_w)
    res = _run(in_maps, T, trace=_trace)
    NTb, NOBp = T // TB, HID // 256
    acc = np.zeros((NTb, NOBp, 128, 2, TB), np.float64)
    for r in res.results:
        acc += r["outT"].astype(np.float64).reshape(NTb, NOBp, 128, 2, TB)
    # untile: out[t, o] with o = (2*obp + oi)*128 + p, t = tb*TB + u
    out = np.ascontiguousarray(
        acc.transpose(0, 4, 1, 3, 2).reshape(T, HID)
    ).astype(np.float32)
    kernel._last_results = res
    return out
